# revision 1
# baseline (speedup 1.0000x reference)
"""Bicubic grid_sample (transpose-like warp) for Trainium2, 8 NeuronCores.

Strategy: shard output rows across cores (256 rows/core). The warp maps
output (i, j) -> input (y ~ j +- 21, x ~ i +- 21), so each core needs an
x-column slab of the image. On device, repack the slab into a patch table
where each 256B unit holds the full 4x4x8ch bicubic patch at (y0, x0)
(fp16, x-replicated). Per 128x128-pixel tile: compute exact floors /
cubic weights on DVE/ACT, bulk dma_gather one 256B patch per pixel,
then weight-multiply + tree-reduce on DVE.
"""
import os, sys, types
sys.path.insert(0, "/opt/trn_rl_repo")
import numpy as np

try:  # register NTFF profile hook so BASS_TRACE=1 can measure HW time
    import antenv
    if "antenv.axon_hooks" not in sys.modules:
        from trn_agent_boot.trn_boot import _ntff_profile_via_ctypes
        _h = _ntff_profile_via_ctypes("/opt/axon/libaxon_pjrt.so")
        _m = types.ModuleType("antenv.axon_hooks")
        _m.get_axon_ntff_profile_hook = lambda: _h
        _m.set_axon_ntff_profile_hook = lambda h: None
        sys.modules["antenv.axon_hooks"] = _m
        antenv.axon_hooks = _m
except Exception:
    pass

import concourse.bass as bass
import concourse.bacc as bacc
import concourse.mybir as mybir
import concourse.tile as tile
from concourse import library_config
from concourse.bass_utils import run_bass_kernel_spmd

F32 = mybir.dt.float32
F16 = mybir.dt.float16
I16 = mybir.dt.int16
I32 = mybir.dt.int32
OP = mybir.AluOpType

N_CORES = 8
H = W = 2048
C = 8
RPC = H // N_CORES          # output rows per core = 256
PAD = 24                    # y halo rows on each side
YS = H + 2 * PAD            # 2096 slab rows
XS = 308                    # slab cols: [I0-24, I0+284)
XT = 176                    # table cols per row-group
YT = YS + 16                # table rows incl. pad so in_ap window stays in-bounds
JW = 128                    # j-chunk width
NJT = W // JW               # 16 j-tiles
A = -0.75                   # bicubic constant

N_JTILES = NJT              # reduced for dev runs if needed


def build_nc():
    nc = bacc.Bacc("TRN2", target_bir_lowering=False, debug=False,
                   num_devices=N_CORES, num_swdge_queues=4)
    xs = nc.dram_tensor("xs", [C, YS + 4, XS], F32, kind="ExternalInput")
    gr = nc.dram_tensor("gr", [RPC, W, 2], F32, kind="ExternalInput")
    out = nc.dram_tensor("out", [C, RPC, W], F32, kind="ExternalOutput")

    with tile.TileContext(nc) as tc:
        nc.gpsimd.load_library(library_config.mlp)
        import contextlib
        with contextlib.ExitStack() as ctx:
            _build_body(ctx, tc, nc, xs, gr, out)
    nc.compile()
    return nc


def _build_body(ctx, tc, nc, xs, gr, out):
    tabpool = ctx.enter_context(tc.tile_pool(name="tab", bufs=1, space="DRAM"))
    bncpool = ctx.enter_context(tc.tile_pool(name="bnc", bufs=4, space="DRAM"))
    gridp = ctx.enter_context(tc.tile_pool(name="grid", bufs=2))
    wrk = ctx.enter_context(tc.tile_pool(name="wrk", bufs=2))
    gp = ctx.enter_context(tc.tile_pool(name="g", bufs=2))
    lp = ctx.enter_context(tc.tile_pool(name="l", bufs=1))
    outp = ctx.enter_context(tc.tile_pool(name="out", bufs=2))

    # two patch tables, one per row-group: [YT*XT units, 128 fp16]
    tabs = []
    for g in range(2):
        tabg = tabpool.tile([YT * XT, 128], F16, tag=f"tab{g}")
        tabs.append(tabg)

    # ---------------- phase 1: repack xs -> tables ----------------
    # table unit (y', xu) elems [s4, r4, c8] = xs[c, y'+r, xu + s + 128*g]
    import contextlib
    with contextlib.ExitStack() as p1ctx:
        repack = p1ctx.enter_context(tc.tile_pool(name="repack", bufs=2))
        tpool = p1ctx.enter_context(tc.tile_pool(name="tgp", bufs=1))
        YB = 124
        n_yb = (YS + YB - 1) // YB
        for yb in range(n_yb):
            y0 = yb * YB
            rows = min(YB, YS - y0)
            tgs = []
            for g in range(2):
                tgt = tpool.tile([128, 179 * 32], F16, tag=f"tg{g}")
                tgs.append(tgt)
            for r in range(4):
                ld = rows
                for c in range(C):
                    t = repack.tile([128, XS], F32, tag=f"xsb{c}")
                    nc.sync.dma_start(t[:ld, :], xs[c, y0 + r:y0 + r + ld, :])
                    for g in range(2):
                        dst = bass.AP(tgs[g].tensor, tgs[g].offset + r * 8 + c,
                                      [[tgs[g].ap[0][0], ld], [32, 179]])
                        src = bass.AP(t.tensor, t.offset + 128 * g,
                                      [[t.ap[0][0], ld], [1, 179]])
                        if (r * C + c) % 2 == 0:
                            nc.vector.tensor_copy(dst, src)
                        else:
                            nc.scalar.copy(dst, src)
            for g in range(2):
                # one DMA writes all 4 s-planes: dst unit = [s,r,c] contiguous
                src = bass.AP(tgs[g].tensor, tgs[g].offset,
                              [[tgs[g].ap[0][0], rows], [32, XT], [32, 4], [1, 32]])
                dst = bass.AP(tabs[g].tensor, tabs[g].offset + y0 * XT * 128,
                              [[XT * 128, rows], [128, XT], [1, 128]])
                nc.sync.dma_start(dst, src)

    # ---------------- phase 2: per-tile gather + combine ----------------
    NI = 128 * JW                     # 16384 idxs per tile
    for g in range(2):
        IG = g * 128                  # row-group base (local to core rows)
        for J in range(N_JTILES):
            jb = J * JW
            ybase = jb                # table row offset for this tile
            # grid tile [128 rows, JW cols, 2]
            gt = gridp.tile([128, JW * 2], F32, tag="gt")
            nc.sync.dma_start(
                gt[:],
                bass.AP(gr, (IG) * W * 2 + jb * 2,
                        [[W * 2, 128], [1, JW * 2]]))
            gx = bass.AP(gt.tensor, gt.offset, [gt.ap[0], [2, JW]])
            gy = bass.AP(gt.tensor, gt.offset + 1, [gt.ap[0], [2, JW]])

            # lx = gx*1024 + (1047.5 - IG); ly = gy*1024 + (1046.5 - ybase)
            lx = wrk.tile([128, JW], F32, tag="lx")
            ly = wrk.tile([128, JW], F32, tag="ly")
            nc.scalar.activation(lx[:], gx, mybir.ActivationFunctionType.Copy,
                                 bias=1047.5 - IG, scale=1024.0)
            nc.scalar.activation(ly[:], gy, mybir.ActivationFunctionType.Copy,
                                 bias=1046.5 - ybase, scale=1024.0)

            def floorpair(v, tag):
                vi = wrk.tile([128, JW], I32, tag=f"vi{tag}")
                nc.vector.tensor_copy(vi[:], v[:])
                vf = wrk.tile([128, JW], F32, tag=f"vf{tag}")
                nc.vector.tensor_copy(vf[:], vi[:])
                co = wrk.tile([128, JW], F32, tag=f"co{tag}")
                nc.vector.tensor_tensor(co[:], vf[:], v[:], op=OP.is_gt)
                nc.vector.tensor_tensor(vf[:], vf[:], co[:], op=OP.subtract)
                fr = wrk.tile([128, JW], F32, tag=f"fr{tag}")
                nc.vector.tensor_tensor(fr[:], v[:], vf[:], op=OP.subtract)
                return vf, fr

            fx, tx = floorpair(lx, "x")   # fx = floor(lx), tx frac
            fy, ty = floorpair(ly, "y")

            # idx = fy*176 + fx - 1
            idxf = wrk.tile([128, JW], F32, tag="idxf")
            nc.vector.scalar_tensor_tensor(idxf[:], fy[:], float(XT), fx[:],
                                           op0=OP.mult, op1=OP.add)
            nc.scalar.activation(idxf[:], idxf[:], mybir.ActivationFunctionType.Copy, bias=-1.0, scale=1.0)
            idx16 = wrk.tile([128, JW], I16, tag="idx16")
            nc.vector.tensor_copy(idx16[:], idxf[:])

            # cubic weights for both dirs: w0..w3 as [128, JW] each
            def cubic(t, tag):
                # w0 = ((A*(t+1) - 5A)*(t+1) + 8A)*(t+1) - 4A
                s0 = wrk.tile([128, JW], F32, tag=f"s0{tag}")
                nc.scalar.activation(s0[:], t[:], mybir.ActivationFunctionType.Copy, bias=1.0, scale=1.0)
                w0 = wrk.tile([128, JW], F32, tag=f"w0{tag}")
                nc.scalar.activation(w0[:], s0[:],
                                     mybir.ActivationFunctionType.Copy,
                                     bias=-5.0 * A, scale=A)
                nc.vector.tensor_tensor(w0[:], w0[:], s0[:], op=OP.mult)
                nc.scalar.activation(w0[:], w0[:], mybir.ActivationFunctionType.Copy, bias=8.0 * A, scale=1.0)
                nc.vector.tensor_tensor(w0[:], w0[:], s0[:], op=OP.mult)
                nc.scalar.activation(w0[:], w0[:], mybir.ActivationFunctionType.Copy, bias=-4.0 * A, scale=1.0)
                # w1 = ((A+2)*t - (A+3))*t*t + 1
                w1 = wrk.tile([128, JW], F32, tag=f"w1{tag}")
                nc.scalar.activation(w1[:], t[:],
                                     mybir.ActivationFunctionType.Copy,
                                     bias=-(A + 3.0), scale=A + 2.0)
                t2 = wrk.tile([128, JW], F32, tag=f"t2{tag}")
                nc.vector.tensor_tensor(t2[:], t[:], t[:], op=OP.mult)
                nc.vector.tensor_tensor(w1[:], w1[:], t2[:], op=OP.mult)
                nc.scalar.activation(w1[:], w1[:], mybir.ActivationFunctionType.Copy, bias=1.0, scale=1.0)
                # u = 1 - t ; w2 = ((A+2)*u - (A+3))*u*u + 1
                u = wrk.tile([128, JW], F32, tag=f"u{tag}")
                nc.scalar.activation(u[:], t[:],
                                     mybir.ActivationFunctionType.Copy,
                                     bias=1.0, scale=-1.0)
                w2 = wrk.tile([128, JW], F32, tag=f"w2{tag}")
                nc.scalar.activation(w2[:], u[:],
                                     mybir.ActivationFunctionType.Copy,
                                     bias=-(A + 3.0), scale=A + 2.0)
                u2 = wrk.tile([128, JW], F32, tag=f"u2{tag}")
                nc.vector.tensor_tensor(u2[:], u[:], u[:], op=OP.mult)
                nc.vector.tensor_tensor(w2[:], w2[:], u2[:], op=OP.mult)
                nc.scalar.activation(w2[:], w2[:], mybir.ActivationFunctionType.Copy, bias=1.0, scale=1.0)
                # w3 = 1 - w0 - w1 - w2
                w3 = wrk.tile([128, JW], F32, tag=f"w3{tag}")
                nc.vector.tensor_tensor(w3[:], w0[:], w1[:], op=OP.add)
                nc.vector.tensor_tensor(w3[:], w3[:], w2[:], op=OP.add)
                nc.scalar.activation(w3[:], w3[:],
                                     mybir.ActivationFunctionType.Copy,
                                     bias=1.0, scale=-1.0)
                return w0, w1, w2, w3

            wx = cubic(tx, "x")
            wy = cubic(ty, "y")

            # pack wx into [128, JW*4] (s-minor), then outer product with wy
            wxp = wrk.tile([128, JW * 4], F32, tag="wxp")
            for s in range(4):
                dst = bass.AP(wxp.tensor, wxp.offset + s, [wxp.ap[0], [4, JW]])
                nc.scalar.copy(dst, wx[s][:])
            wp = wrk.tile([128, JW * 16], F16, tag="wp")
            for r in range(4):
                # wp[.., jj, s, r] = wxp[jj, s] * wy_r[jj]
                dst = bass.AP(wp.tensor, wp.offset + r,
                              [wp.ap[0], [16, JW], [4, 4]])
                src0 = bass.AP(wxp.tensor, wxp.offset,
                               [wxp.ap[0], [4, JW], [1, 4]])
                src1 = bass.AP(wy[r].tensor, wy[r].offset,
                               [wy[r].ap[0], [1, JW], [0, 4]])
                nc.vector.tensor_tensor(dst, src0, src1, op=OP.mult)

            # bounce idx to DRAM, read back wrapped+replicated
            bnc = bncpool.tile([128 * JW], I16, tag="bnc")
            nc.sync.dma_start(
                bass.AP(bnc.tensor, bnc.offset, [[JW, 128], [1, JW]]),
                idx16[:])
            idxw = wrk.tile([128, JW * 8], I16, tag="idxw")
            for k in range(8):
                nc.sync.dma_start(
                    bass.AP(idxw.tensor, idxw.offset + k,
                            [[idxw.ap[0][0], 16], [8, JW], [1, 1]]),
                    bass.AP(bnc.tensor, bnc.offset + k * 16 * JW,
                            [[JW, 16], [1, JW], [1, 1]]))
            for rep in range(1, 8):
                nc.sync.dma_start(
                    bass.AP(idxw.tensor, idxw.offset + 16 * rep * idxw.ap[0][0],
                            [[idxw.ap[0][0], 16], [1, JW * 8]]),
                    bass.AP(idxw.tensor, idxw.offset,
                            [[idxw.ap[0][0], 16], [1, JW * 8]]))

            # bulk gather: 16384 patches of 256B
            G = gp.tile([128, JW, 128], F16, tag="G")
            in_ap = bass.AP(tabs[g].tensor,
                            tabs[g].offset + ybase * XT * 128,
                            [[128, 186 * XT], [1, 128]])
            NSUB = 4096
            for m in range(NI // NSUB):
                nc.gpsimd.dma_gather(
                    out_ap=G[:, m * (NSUB // 128):(m + 1) * (NSUB // 128), :],
                    in_ap=in_ap,
                    idxs_ap=idxw[:, m * (NSUB // 16):(m + 1) * (NSUB // 16)],
                    num_idxs=NSUB,
                    num_idxs_reg=NSUB,
                    elem_size=128,
                    elem_step=128,
                    single_packet=False,
                    queue_num=(g * N_JTILES * 4 + J * 4 + m) % 4,
                )

            if os.environ.get("KDBG") and g == 0 and J == 0:
                dbgG = nc.dram_tensor("dbgG", [128, JW * 128], F16, kind="ExternalOutput")
                nc.sync.dma_start(bass.AP(dbgG, 0, [[JW * 128, 128], [1, JW * 128]]),
                                  bass.AP(G.tensor, G.offset, [G.ap[0], [1, JW * 128]]))
                dbgW = nc.dram_tensor("dbgW", [128, JW * 16], F16, kind="ExternalOutput")
                nc.sync.dma_start(bass.AP(dbgW, 0, [[JW * 16, 128], [1, JW * 16]]), wp[:])
                dbgI = nc.dram_tensor("dbgI", [128, JW * 8], I16, kind="ExternalOutput")
                nc.sync.dma_start(bass.AP(dbgI, 0, [[JW * 8, 128], [1, JW * 8]]), idxw[:])
            # combine: P = G * wp (bcast over c) in-place, tree-reduce, out f32
            src1 = bass.AP(wp.tensor, wp.offset,
                           [wp.ap[0], [16, JW], [4, 4], [1, 4], [0, 8]])
            src0 = bass.AP(G.tensor, G.offset,
                           [G.ap[0], [128, JW], [32, 4], [8, 4], [1, 8]])
            nc.vector.tensor_tensor(src0, src0, src1, op=OP.mult)
            P = G

            # reduce over s (stride 32): 4 -> 2 -> 1, then r (stride 8)
            def halve(buf, npx, stride, n, tag):
                # adds pairs along the dim with given stride/count n -> n/2
                o = lp.tile([128, JW * stride * (n // 2)], F16, tag=tag)
                i0 = bass.AP(buf.tensor, buf.offset,
                             [buf.ap[0], [stride * n, npx], [stride * 2, n // 2], [1, stride]])
                i1 = bass.AP(buf.tensor, buf.offset + stride,
                             [buf.ap[0], [stride * n, npx], [stride * 2, n // 2], [1, stride]])
                od = bass.AP(o.tensor, o.offset,
                             [o.ap[0], [stride * (n // 2), npx], [stride, n // 2], [1, stride]])
                nc.vector.tensor_tensor(od, i0, i1, op=OP.add)
                return o

            L1 = halve(P, JW, 32, 4, "L1")     # sum s pairs -> [jj, 2, r, c](64)
            L2 = halve(L1, JW, 32, 2, "L2")    # -> [jj, r4, c8](32)
            L3 = halve(L2, JW, 8, 4, "L3")     # sum r pairs -> [jj, 2, c](16)
            # final level: write f32 transposed to (c, jj)
            of = outp.tile([128, 8 * JW], F32, tag="of")
            i0 = bass.AP(L3.tensor, L3.offset, [L3.ap[0], [16, JW], [1, 8]])
            i1 = bass.AP(L3.tensor, L3.offset + 8, [L3.ap[0], [16, JW], [1, 8]])
            od = bass.AP(of.tensor, of.offset, [of.ap[0], [1, JW], [JW, 8]])
            nc.vector.tensor_tensor(od, i0, i1, op=OP.add)

            # write out[c, IG+p, jb:jb+JW]
            dsto = bass.AP(out, IG * W + jb,
                           [[W, 128], [RPC * W, 8], [1, JW]])
            nc.sync.dma_start(dsto, of[:])


_NC_CACHE = None


def kernel(x: np.ndarray, grid: np.ndarray) -> np.ndarray:
    global _NC_CACHE
    if _NC_CACHE is None:
        _NC_CACHE = build_nc()
    nc = _NC_CACHE

    x0 = np.ascontiguousarray(x[0], dtype=np.float32)        # [C, H, W]
    g0 = np.ascontiguousarray(grid[0], dtype=np.float32)     # [H, W, 2]

    in_maps = []
    for k in range(N_CORES):
        I0 = k * RPC
        xsl = np.zeros((C, YS + 4, XS), dtype=np.float32)
        c0 = I0 - PAD
        lo, hi = max(0, c0), min(W, c0 + XS)
        xsl[:, PAD:PAD + H, lo - c0:hi - c0] = x0[:, :, lo:hi]
        grc = np.ascontiguousarray(g0[I0:I0 + RPC]).copy()
        grc[..., 0] -= I0 / 1024.0   # fold per-core x-base into gx
        in_maps.append({"xs": xsl, "gr": grc})

    res = run_bass_kernel_spmd(nc, in_maps, core_ids=list(range(N_CORES)),
                               trace=False)
    global _LAST_EXEC_NS
    _LAST_EXEC_NS = res.exec_time_ns
    out = np.empty((1, C, H, W), dtype=np.float32)
    for k in range(N_CORES):
        out[0, :, k * RPC:(k + 1) * RPC, :] = res.results[k]["out"]
    return out



# revision 6
# speedup vs baseline: 1.3098x; 1.3098x over previous
"""Bicubic grid_sample (transpose-like warp) for Trainium2, 8 NeuronCores.

Strategy: shard output rows across cores (256 rows/core). The warp maps
output (i, j) -> input (y ~ j +- 21, x ~ i +- 21), so each core needs an
x-column slab of the image. On device, repack the slab into a patch table
in DRAM where each 256B unit holds the full 4x4x8ch bicubic patch at
(y0, x0) (fp16). v2: the table is built from a fully s,r-materialized
SBUF staging buffer and written via SWDGE with ~22KB contiguous
descriptors (16-engine spread); indices are folded into the gather's
wrapped 16-partition layout on-chip (no DRAM bounce); weights are
computed on 512-wide super-tiles; row-group 1's table build is emitted
interleaved with row-group 0's gather/combine so they overlap.
"""
import os, sys, types
sys.path.insert(0, "/opt/trn_rl_repo")
import numpy as np

try:  # register NTFF profile hook so BASS_TRACE=1 can measure HW time
    import antenv
    if "antenv.axon_hooks" not in sys.modules:
        from trn_agent_boot.trn_boot import _ntff_profile_via_ctypes
        _h = _ntff_profile_via_ctypes("/opt/axon/libaxon_pjrt.so")
        _m = types.ModuleType("antenv.axon_hooks")
        _m.get_axon_ntff_profile_hook = lambda: _h
        _m.set_axon_ntff_profile_hook = lambda h: None
        sys.modules["antenv.axon_hooks"] = _m
        antenv.axon_hooks = _m
except Exception:
    pass

import concourse.bass as bass
import concourse.bacc as bacc
import concourse.mybir as mybir
import concourse.tile as tile
from concourse import library_config
from concourse.bass_utils import run_bass_kernel_spmd

F32 = mybir.dt.float32
F16 = mybir.dt.float16
I16 = mybir.dt.int16
I32 = mybir.dt.int32
OP = mybir.AluOpType
ACTF = None  # set after import

N_CORES = 8
H = W = 2048
C = 8
RPC = H // N_CORES          # output rows per core = 256
PAD = 24                    # y halo rows on each side
YS = H + 2 * PAD            # 2096 slab rows
XS = 308                    # slab cols: [I0-24, I0+284)
XT = 176                    # table cols per row-group
XH = 88                     # x-half of the table staging buffer
YT = YS + 16                # table rows incl. pad so in_ap window stays in-bounds
SJW = 512                   # super-tile width (weights/idx granularity)
JW2 = 64                    # half-tile width (gather/combine granularity)
A = -0.75                   # bicubic constant
YB = 124                    # y-block rows for table build
N_YB = (YS + YB - 1) // YB  # 17


def build_nc():
    nc = bacc.Bacc("TRN2", target_bir_lowering=False, debug=False,
                   num_devices=N_CORES, num_swdge_queues=4)
    xs = nc.dram_tensor("xs", [C, YS + 4, XS], F32, kind="ExternalInput")
    gr = nc.dram_tensor("gr", [RPC, W, 2], F32, kind="ExternalInput")
    out = nc.dram_tensor("out", [C, RPC, W], F32, kind="ExternalOutput")

    with tile.TileContext(nc) as tc:
        nc.gpsimd.load_library(library_config.mlp)
        import contextlib
        with contextlib.ExitStack() as ctx:
            _build_body(ctx, tc, nc, xs, gr, out)
    nc.compile()
    return nc


def _build_body(ctx, tc, nc, xs, gr, out):
    Copy = mybir.ActivationFunctionType.Copy
    tabpool = ctx.enter_context(tc.tile_pool(name="tab", bufs=1, space="DRAM"))
    # phase-1 pools
    tpool = ctx.enter_context(tc.tile_pool(name="t", bufs=2))
    tgpool = ctx.enter_context(tc.tile_pool(name="tg", bufs=1))
    tg2pool = ctx.enter_context(tc.tile_pool(name="tg2", bufs=1))
    # phase-2 pools
    gridp = ctx.enter_context(tc.tile_pool(name="grid", bufs=2))
    wrk = ctx.enter_context(tc.tile_pool(name="wrk", bufs=1))
    wpp = ctx.enter_context(tc.tile_pool(name="wpp", bufs=2))
    idxp = ctx.enter_context(tc.tile_pool(name="idx", bufs=2))
    idxs1 = ctx.enter_context(tc.tile_pool(name="idx1", bufs=1))
    gp = ctx.enter_context(tc.tile_pool(name="g", bufs=2))
    lp = ctx.enter_context(tc.tile_pool(name="l", bufs=1))
    outp = ctx.enter_context(tc.tile_pool(name="out", bufs=2))

    tabs = []
    for g in range(2):
        tabg = tabpool.tile([YT * XT, 128], F16, tag=f"tab{g}")
        tabs.append(tabg)

    hwdge = [nc.sync, nc.scalar]
    cnt = {"dma": 0, "cp": 0, "q": 0}

    def eng():
        cnt["dma"] += 1
        return hwdge[cnt["dma"] % 2]

    def ccopy(dst, src):
        cnt["cp"] += 1
        if cnt["cp"] % 2 == 0:
            nc.vector.tensor_copy(dst, src)
        else:
            nc.scalar.copy(dst, src)

    # ---------------- phase 1: repack xs -> table[g], one y-block ----------
    def build_block(g, yb):
        y0 = yb * YB
        rows = min(YB, YS - y0)
        tg = tgpool.tile([128, 179 * 32], F16, tag="tg")
        for r in range(4):
            for c in range(C):
                t = tpool.tile([128, 179], F32, tag=f"xsb{c}")
                eng().dma_start(
                    t[:rows, :],
                    bass.AP(xs, c * (YS + 4) * XS + (y0 + r) * XS + 128 * g,
                            [[XS, rows], [1, 179]]))
                dst = bass.AP(tg.tensor, tg.offset + r * 8 + c,
                              [[tg.ap[0][0], rows], [32, 179]])
                ccopy(dst, t[:rows, :])
        for h in range(2):
            tg2 = tg2pool.tile([128, XH * 128], F16, tag=f"tg2{h}")
            for s in range(4):
                src = bass.AP(tg.tensor, tg.offset + (XH * h + s) * 32,
                              [[tg.ap[0][0], rows], [32, XH], [1, 32]])
                dst = bass.AP(tg2.tensor, tg2.offset + s * 32,
                              [[tg2.ap[0][0], rows], [128, XH], [1, 32]])
                ccopy(dst, src)
            # one SWDGE DMA, contiguous 22.5KB per row on both sides
            dsta = bass.AP(tabs[g].tensor,
                           tabs[g].offset + (y0 * XT + h * XH) * 128,
                           [[XT * 128, rows], [1, XH * 128]])
            nc.gpsimd.dma_start(dsta, tg2[:rows, :])

    # ---------------- phase 2: per super-tile weights+idx, gather+combine --
    def cubic(t, tag, outdt):
        # returns w0..w3 tiles [128, SJW] in outdt; scratch tags shared
        # between calls (sequential use).
        s0 = wrk.tile([128, SJW], F32, tag="c_s0")
        nc.scalar.activation(s0[:], t[:], Copy, bias=1.0, scale=1.0)
        w0f = wrk.tile([128, SJW], F32, tag="c_w0f")
        nc.scalar.activation(w0f[:], s0[:], Copy, bias=-5.0 * A, scale=A)
        nc.vector.tensor_tensor(w0f[:], w0f[:], s0[:], op=OP.mult)
        nc.scalar.activation(w0f[:], w0f[:], Copy, bias=8.0 * A, scale=1.0)
        nc.vector.tensor_tensor(w0f[:], w0f[:], s0[:], op=OP.mult)
        w0 = wrk.tile([128, SJW], outdt, tag=f"w0{tag}")
        nc.scalar.activation(w0[:], w0f[:], Copy, bias=-4.0 * A, scale=1.0)
        # w1
        w1f = wrk.tile([128, SJW], F32, tag="c_w1f")
        nc.scalar.activation(w1f[:], t[:], Copy, bias=-(A + 3.0), scale=A + 2.0)
        t2 = wrk.tile([128, SJW], F32, tag="c_t2")
        nc.vector.tensor_tensor(t2[:], t[:], t[:], op=OP.mult)
        nc.vector.tensor_tensor(w1f[:], w1f[:], t2[:], op=OP.mult)
        w1 = wrk.tile([128, SJW], outdt, tag=f"w1{tag}")
        nc.scalar.activation(w1[:], w1f[:], Copy, bias=1.0, scale=1.0)
        # w2: u = 1 - t
        u = wrk.tile([128, SJW], F32, tag="c_u")
        nc.scalar.activation(u[:], t[:], Copy, bias=1.0, scale=-1.0)
        w2f = wrk.tile([128, SJW], F32, tag="c_w2f")
        nc.scalar.activation(w2f[:], u[:], Copy, bias=-(A + 3.0), scale=A + 2.0)
        u2 = wrk.tile([128, SJW], F32, tag="c_u2")
        nc.vector.tensor_tensor(u2[:], u[:], u[:], op=OP.mult)
        nc.vector.tensor_tensor(w2f[:], w2f[:], u2[:], op=OP.mult)
        w2 = wrk.tile([128, SJW], outdt, tag=f"w2{tag}")
        nc.scalar.activation(w2[:], w2f[:], Copy, bias=1.0, scale=1.0)
        # w3 = 1 - w0 - w1 - w2 (in f32 then cast)
        w3f = wrk.tile([128, SJW], F32, tag="c_w3f")
        nc.vector.tensor_tensor(w3f[:], w0[:], w1[:], op=OP.add)
        nc.vector.tensor_tensor(w3f[:], w3f[:], w2[:], op=OP.add)
        w3 = wrk.tile([128, SJW], outdt, tag=f"w3{tag}")
        nc.scalar.activation(w3[:], w3f[:], Copy, bias=1.0, scale=-1.0)
        return [w0, w1, w2, w3]

    def floorpair(v, tag):
        # vi/co scratch shared between calls; vf/fr persist per-dir
        vi = wrk.tile([128, SJW], I32, tag="f_vi")
        nc.vector.tensor_copy(vi[:], v[:])
        vf = wrk.tile([128, SJW], F32, tag=f"vf{tag}")
        nc.vector.tensor_copy(vf[:], vi[:])
        co = wrk.tile([128, SJW], F32, tag="f_co")
        nc.vector.tensor_tensor(co[:], vf[:], v[:], op=OP.is_gt)
        nc.vector.tensor_tensor(vf[:], vf[:], co[:], op=OP.subtract)
        fr = wrk.tile([128, SJW], F32, tag=f"fr{tag}")
        nc.vector.tensor_tensor(fr[:], v[:], vf[:], op=OP.subtract)
        return vf, fr

    def super_tile(g, s4):
        """Weights + wrapped idx for 512 output cols of row-group g.
        Returns (wxp, wy, C_idx) tiles."""
        IG = g * 128
        jb4 = s4 * SJW
        gt = gridp.tile([128, SJW * 2], F32, tag="gt")
        eng().dma_start(
            gt[:],
            bass.AP(gr, IG * W * 2 + jb4 * 2, [[W * 2, 128], [1, SJW * 2]]))
        gx = bass.AP(gt.tensor, gt.offset, [gt.ap[0], [2, SJW]])
        gy = bass.AP(gt.tensor, gt.offset + 1, [gt.ap[0], [2, SJW]])

        lx = wrk.tile([128, SJW], F32, tag="lx")
        ly = wrk.tile([128, SJW], F32, tag="ly")
        nc.scalar.activation(lx[:], gx, Copy, bias=1047.5 - IG, scale=1024.0)
        nc.scalar.activation(ly[:], gy, Copy, bias=1046.5 - jb4, scale=1024.0)
        fx, tx = floorpair(lx, "x")
        fy, ty = floorpair(ly, "y")

        # idxf = fy*XT + fx - 1 (f32, exact)
        idxf = wrk.tile([128, SJW], F32, tag="idxf")
        nc.vector.scalar_tensor_tensor(idxf[:], fy[:], float(XT), fx[:],
                                       op0=OP.mult, op1=OP.add)
        # per-sub-tile rebase to the 186-row gather window, cast to i16
        idx16 = idxs1.tile([128, SJW], I16, tag="idx16")
        for t in range(SJW // 128):
            nc.vector.tensor_scalar(
                bass.AP(idx16.tensor, idx16.offset + t * 128,
                        [[idx16.ap[0][0], 128], [1, 128]]),
                bass.AP(idxf.tensor, idxf.offset + t * 128,
                        [[idxf.ap[0][0], 128], [1, 128]]),
                -1.0 - t * 128.0 * XT, None, op0=OP.add)

        # fold [128, SJW] -> wrapped [16, 8*SJW]: D[p, k*SJW + j] = idx16[16k+p, j]
        D = idxs1.tile([128, 8 * SJW], I16, tag="D")
        for k in range(8):
            src = bass.AP(idx16.tensor,
                          idx16.offset + 16 * k * idx16.ap[0][0],
                          [[idx16.ap[0][0], 16], [1, SJW]])
            dst = bass.AP(D.tensor, D.offset + k * SJW,
                          [[D.ap[0][0], 16], [1, SJW]])
            eng().dma_start(dst, src)
        # interleave: Cw[p, 8j+k] = D[p, k*SJW + j]
        Cw = idxp.tile([128, 8 * SJW], I16, tag="Cw")
        for k in range(8):
            src = bass.AP(D.tensor, D.offset + k * SJW,
                          [[D.ap[0][0], 16], [1, SJW]])
            dst = bass.AP(Cw.tensor, Cw.offset + k,
                          [[Cw.ap[0][0], 16], [8, SJW]])
            nc.vector.tensor_copy(dst, src)
        # replicate to all 8 gpsimd cores
        for rep in range(1, 8):
            src = bass.AP(Cw.tensor, Cw.offset, [[Cw.ap[0][0], 16], [1, 8 * SJW]])
            dst = bass.AP(Cw.tensor, Cw.offset + 16 * rep * Cw.ap[0][0],
                          [[Cw.ap[0][0], 16], [1, 8 * SJW]])
            eng().dma_start(dst, src)

        wx = cubic(tx, "x", F32)
        wy = cubic(ty, "y", F16)
        # wxp[j, s] packed s-minor, f16
        wxp = wpp.tile([128, SJW * 4], F16, tag="wxp")
        for s in range(4):
            dst = bass.AP(wxp.tensor, wxp.offset + s, [wxp.ap[0], [4, SJW]])
            nc.scalar.copy(dst, wx[s][:])
        return wxp, wy, Cw

    def half_tile(g, s4, t, h, wxp, wy, Cw):
        IG = g * 128
        jb = s4 * SJW + t * 128 + h * JW2
        ybase = s4 * SJW + t * 128
        # wp_h[j, s, r] = wxp[j, s] * wy_r[j]  (JW2 cols)
        joff = t * 128 + h * JW2
        wp = wpp.tile([128, JW2 * 16], F16, tag="wp")
        for r in range(4):
            dst = bass.AP(wp.tensor, wp.offset + r,
                          [wp.ap[0], [16, JW2], [4, 4]])
            src0 = bass.AP(wxp.tensor, wxp.offset + joff * 4,
                           [wxp.ap[0], [4, JW2], [1, 4]])
            src1 = bass.AP(wy[r].tensor, wy[r].offset + joff,
                           [wy[r].ap[0], [1, JW2], [0, 4]])
            nc.vector.tensor_tensor(dst, src0, src1, op=OP.mult)

        NI = 128 * JW2  # 8192
        G = gp.tile([128, JW2 * 128], F16, tag="G")
        in_ap = bass.AP(tabs[g].tensor,
                        tabs[g].offset + ybase * XT * 128,
                        [[128, 186 * XT], [1, 128]])
        idxs = bass.AP(Cw.tensor, Cw.offset + (t * 128 + h * JW2) * 8,
                       [[Cw.ap[0][0], 128], [1, NI // 16]])
        q = cnt["q"] % 4
        cnt["q"] += 1
        nc.gpsimd.dma_gather(
            out_ap=bass.AP(G.tensor, G.offset,
                           [[G.ap[0][0], 128], [128, JW2], [1, 128]]),
            in_ap=in_ap,
            idxs_ap=idxs,
            num_idxs=NI,
            num_idxs_reg=NI,
            elem_size=128,
            elem_step=128,
            single_packet=False,
            queue_num=q,
        )

        # combine: P = G * wp (bcast over c), tree-reduce s then r
        src1 = bass.AP(wp.tensor, wp.offset,
                       [wp.ap[0], [16, JW2], [4, 4], [1, 4], [0, 8]])
        src0 = bass.AP(G.tensor, G.offset,
                       [G.ap[0], [128, JW2], [32, 4], [8, 4], [1, 8]])
        nc.vector.tensor_tensor(src0, src0, src1, op=OP.mult)

        def halve(buf, stride, n, tag, npx=JW2):
            o = lp.tile([128, npx * stride * (n // 2)], F16, tag=tag)
            i0 = bass.AP(buf.tensor, buf.offset,
                         [buf.ap[0], [stride * n, npx], [stride * 2, n // 2], [1, stride]])
            i1 = bass.AP(buf.tensor, buf.offset + stride,
                         [buf.ap[0], [stride * n, npx], [stride * 2, n // 2], [1, stride]])
            od = bass.AP(o.tensor, o.offset,
                         [o.ap[0], [stride * (n // 2), npx], [stride, n // 2], [1, stride]])
            nc.vector.tensor_tensor(od, i0, i1, op=OP.add)
            return o

        L1 = halve(G, 32, 4, "L1")
        L2 = halve(L1, 32, 2, "L2")
        L3 = halve(L2, 8, 4, "L3")
        of = outp.tile([128, 8 * JW2], F32, tag="of")
        i0 = bass.AP(L3.tensor, L3.offset, [L3.ap[0], [16, JW2], [1, 8]])
        i1 = bass.AP(L3.tensor, L3.offset + 8, [L3.ap[0], [16, JW2], [1, 8]])
        od = bass.AP(of.tensor, of.offset, [of.ap[0], [1, JW2], [JW2, 8]])
        nc.vector.tensor_tensor(od, i0, i1, op=OP.add)

        dsto = bass.AP(out, IG * W + jb,
                       [[W, 128], [RPC * W, 8], [1, JW2]])
        eng().dma_start(dsto, of[:])

    # ---------------- emission schedule --------------------------------
    for yb in range(N_YB):
        build_block(0, yb)

    # run g0 interleaved with build of g1
    runs = [(s4, t, h) for s4 in range(4) for t in range(4) for h in range(2)]
    built = 0
    sup = None
    for i, (s4, t, h) in enumerate(runs):
        if t == 0 and h == 0:
            sup = super_tile(0, s4)
        half_tile(0, s4, t, h, *sup)
        want = (i + 1) * N_YB // len(runs)
        while built < want:
            build_block(1, built)
            built += 1
    while built < N_YB:
        build_block(1, built)
        built += 1
    for s4 in range(4):
        sup = super_tile(1, s4)
        for t in range(4):
            for h in range(2):
                half_tile(1, s4, t, h, *sup)


_NC_CACHE = None


def kernel(x: np.ndarray, grid: np.ndarray) -> np.ndarray:
    global _NC_CACHE
    if _NC_CACHE is None:
        _NC_CACHE = build_nc()
    nc = _NC_CACHE

    x0 = np.ascontiguousarray(x[0], dtype=np.float32)        # [C, H, W]
    g0 = np.ascontiguousarray(grid[0], dtype=np.float32)     # [H, W, 2]

    in_maps = []
    for k in range(N_CORES):
        I0 = k * RPC
        xsl = np.zeros((C, YS + 4, XS), dtype=np.float32)
        c0 = I0 - PAD
        lo, hi = max(0, c0), min(W, c0 + XS)
        xsl[:, PAD:PAD + H, lo - c0:hi - c0] = x0[:, :, lo:hi]
        grc = np.ascontiguousarray(g0[I0:I0 + RPC]).copy()
        grc[..., 0] -= I0 / 1024.0   # fold per-core x-base into gx
        in_maps.append({"xs": xsl, "gr": grc})

    res = run_bass_kernel_spmd(nc, in_maps, core_ids=list(range(N_CORES)),
                               trace=False)
    global _LAST_EXEC_NS
    _LAST_EXEC_NS = res.exec_time_ns
    out = np.empty((1, C, H, W), dtype=np.float32)
    for k in range(N_CORES):
        out[0, :, k * RPC:(k + 1) * RPC, :] = res.results[k]["out"]
    return out
